# revision 30
# baseline (speedup 1.0000x reference)
"""Trainium2 Bass kernel for GQA attention block (B=2, S=2048, HS=2048, H=16, HKV=4, D=128).

Strategy (8 NeuronCores, SPMD):
  - Head-parallel: core c computes q-heads {2c, 2c+1} and kv-head c//2 for BOTH batches.
  - bf16 matmuls throughout (fp8 fails the 2e-2 accuracy gate: random-sign dot
    products keep the ~3% element quantization error instead of averaging it).
  - RMS-norm + RoPE computed straight from PSUM: RoPE first (rotation preserves
    norms), then a single broadcast multiply by rsqrt(mean-square). Work spread
    across Vector/Scalar/GpSimd so no single engine saturates.
  - Causal flash attention in transposed layout: S^T = K_rope @ Q_rope^T ([kv, q]),
    additive causal masks applied in PSUM before exp, exp on ScalarE over two
    kv-blocks at once, diagonal-group QK matmuls skip fully-masked columns,
    O^T = V^T @ P^T and denominators via ones-matmul accumulated in PSUM.
  - One 8-rank AllToAll per q-head redistributes head-shards -> (batch, seq-strip)
    shards; head 0's collective overlaps head 1's attention.
  - Output projection per strip with a fused add epilogue; host concatenates strips.
"""

import sys

sys.path.insert(0, "/opt/trn_rl_repo")

import numpy as np
import ml_dtypes

BF16 = ml_dtypes.bfloat16

B, H, HKV, D = 2, 16, 4, 128
EPS = 1e-6
P = 128
N_CORES = 8
NEGM = -60.0


def build(S=2048, HS=2048, probe="full"):
    """Build + compile the SPMD graph. Returns the Bacc module."""
    import concourse.bacc as bacc
    import concourse.tile as tile
    import concourse.mybir as mybir

    dt = mybir.dt
    f32 = dt.float32
    bf16 = dt.bfloat16
    AF = mybir.ActivationFunctionType
    ALU = mybir.AluOpType

    T = S // P          # tok tiles per batch (16)
    M = 2 * T           # tok tiles total (2 batches)
    KT = HS // P        # contraction tiles for qkv projection (16)
    KO = (H * D) // P   # contraction tiles for o projection (16)
    CW = S // 4         # q-chunk width == strip width (512)
    CB = CW // P        # kv blocks per chunk step (4)
    NQ = 2              # q heads per core

    nc = bacc.Bacc("TRN2", target_bir_lowering=False, debug=False,
                   enable_asserts=True, num_devices=N_CORES)

    xT = nc.dram_tensor("xT", [M, P, KT * P], bf16, kind="ExternalInput")
    wqkvT = nc.dram_tensor("wqkvT", [P, KT * 512], bf16, kind="ExternalInput")
    woT = nc.dram_tensor("woT", [P, KO * HS], bf16, kind="ExternalInput")
    cos3_d = nc.dram_tensor("cos3", [P, T * 384], bf16, kind="ExternalInput")
    sin3_d = nc.dram_tensor("sin3", [P, T * 384], bf16, kind="ExternalInput")
    maskA_d = nc.dram_tensor("maskA", [P, 2 * 256], f32, kind="ExternalInput")
    maskB_d = nc.dram_tensor("maskB", [P, 2 * 512], f32, kind="ExternalInput")
    onesq_d = nc.dram_tensor("onesq", [P, P], bf16, kind="ExternalInput")
    ident_d = nc.dram_tensor("ident", [P, P], bf16, kind="ExternalInput")
    out_d = nc.dram_tensor("out", [CW, HS], f32, kind="ExternalOutput")

    with tile.TileContext(nc) as tc:
        with tc.tile_pool(name="const", bufs=1) as cpool, \
             tc.tile_pool(name="dram", bufs=1, space="DRAM") as dpool:

            # o-projection weights: allocated up front (outlives stageA pools);
            # the DMAs are issued after stage 1+2 so they don't delay the
            # x / wqkv loads feeding the first matmuls
            wo_sb, _wo_free = tc.tile([P, KO, HS], bf16, name="wo_sb")

            # pools that live only through stages 1-3 (freed before o-proj)
            stageA = tc.tile_pool(name="stageA", bufs=1)
            qkvpool = stageA.__enter__()
            wq_cm = tc.tile_pool(name="wqp", bufs=1)
            wqpool = wq_cm.__enter__()
            xin_cm = tc.tile_pool(name="xin", bufs=4)
            xin = xin_cm.__enter__()

            xms = {}

            def load_xm(m):
                t_ = xin.tile([P, KT, P], bf16, tag="xm", name=f"xm{m}")
                src = xT.ap()[m].rearrange("p (k t) -> p k t", k=KT)
                if m == 0:
                    # finer pieces so the first matmuls can start sooner
                    for k4 in range(0, KT, 4):
                        nc.sync.dma_start(t_[:, k4:k4 + 4, :], src[:, k4:k4 + 4, :])
                else:
                    nc.sync.dma_start(t_[:], src)
                xms[m] = t_

            wqkv_sb = wqpool.tile([P, KT, 512], bf16, name="wqkv_sb")
            wq_src = wqkvT.ap().rearrange("p (k f) -> p k f", k=KT)
            load_xm(0)
            nc.sync.dma_start(wqkv_sb[:, 0:4, :], wq_src[:, 0:4, :])
            load_xm(1)
            for k4 in range(4, KT, 4):
                nc.sync.dma_start(wqkv_sb[:, k4:k4 + 4, :], wq_src[:, k4:k4 + 4, :])
            load_xm(2)

            cos3_sb = cpool.tile([P, T, 384], bf16, name="cos3_sb")
            sin3_sb = cpool.tile([P, T, 384], bf16, name="sin3_sb")
            nc.sync.dma_start(cos3_sb[:], cos3_d.ap().rearrange("p (t d) -> p t d", t=T))
            nc.sync.dma_start(sin3_sb[:], sin3_d.ap().rearrange("p (t d) -> p t d", t=T))
            maskA_sb = cpool.tile([P, 2, 256], f32, name="maskA_sb")
            nc.sync.dma_start(maskA_sb[:], maskA_d.ap().rearrange("p (i t) -> p i t", i=2))
            maskB_sb = cpool.tile([P, 2, 512], f32, name="maskB_sb")
            nc.sync.dma_start(maskB_sb[:], maskB_d.ap().rearrange("p (i t) -> p i t", i=2))
            onesq_sb = cpool.tile([P, P], bf16, name="onesq_sb")
            nc.sync.dma_start(onesq_sb[:], onesq_d.ap())
            ident_sb = cpool.tile([P, P], bf16, name="ident_sb")
            nc.sync.dma_start(ident_sb[:], ident_d.ap())
            eps_sb = cpool.tile([P, 1], f32, name="eps_sb")
            nc.gpsimd.memset(eps_sb[:], EPS)

            a2a_in = [dpool.tile([1024, CW], bf16, name=f"a2a_in{h}")
                      for h in range(NQ)]
            a2a_out = [dpool.tile([1024, CW], bf16, name=f"a2a_out{h}")
                       for h in range(NQ)]

            qT_sb = qkvpool.tile([P, NQ, 2 * S], bf16, name="qT_sb")
            kT_sb = qkvpool.tile([P, 2 * S], bf16, name="kT_sb")
            v_sb = qkvpool.tile([P, M, D], bf16, name="v_sb")
            qs_all = qkvpool.tile([P, M, 384], bf16, name="qs_all")

            attn_sb = [cpool.tile([P, KO // NQ, CW], bf16, name=f"attn_sb{h}")
                       for h in range(NQ)]

            # ---------------- stage 1+2: QKV projection, RoPE, RMS norm, transpose
            with tc.tile_pool(name="s12", bufs=3) as s12, \
                 tc.tile_pool(name="ps12", bufs=3, space="PSUM") as ps12:

                def transpose_m(mt):
                    # transposes of qs_all[mt] interleave with later m's QKV
                    # matmuls; their input has been ready for several
                    # iterations so the PE never stalls on them
                    col = P * mt if mt < T else S + P * (mt % T)
                    tp3 = ps12.tile([P, 3, P], bf16, tag="tp", name=f"tp{mt}")
                    for idx in range(3):
                        nc.tensor.transpose(tp3[:, idx, :],
                                            qs_all[:, mt, idx * P:(idx + 1) * P],
                                            ident_sb)
                    nc.vector.tensor_scalar_add(
                        qT_sb[:, 0:2, col:col + P], tp3[:, 0:2, :], 0.0)
                    nc.scalar.copy(kT_sb[:, col:col + P], tp3[:, 2, :])

                for m in range(M):
                    mm = m % T
                    if m + 3 < M:
                        load_xm(m + 3)
                    xm = xms.pop(m)
                    ps_qkv = ps12.tile([P, 512], f32, tag="qkv")
                    for k in range(KT):
                        nc.tensor.matmul(ps_qkv, xm[:, k, :], wqkv_sb[:, k, :],
                                         start=(k == 0), stop=(k == KT - 1))
                    # V: copy to [tok, d] layout
                    nc.scalar.copy(v_sb[:, m, :], ps_qkv[:, 384:512])
                    # sum of squares per head (q0|q1|k) for RMS norm
                    sq = s12.tile([P, 384], bf16, tag="sq")
                    nc.scalar.square(sq, ps_qkv[:, 0:384])
                    ssum = s12.tile([P, 3], f32, tag="ssum")
                    nc.vector.tensor_reduce(
                        ssum, sq.rearrange("p (g d) -> p g d", g=3),
                        mybir.AxisListType.X, ALU.add)
                    rms = s12.tile([P, 3], f32, tag="rms")
                    nc.scalar.activation(rms, ssum, AF.Sqrt,
                                         bias=eps_sb[:], scale=1.0 / D)
                    rinv = s12.tile([P, 3], f32, tag="rinv")
                    nc.vector.reciprocal_approx_fast(rinv, rms)
                    # RoPE on raw PSUM values (rotation commutes with the norm)
                    ro = s12.tile([P, 384], bf16, tag="ro")
                    nc.vector.tensor_tensor(ro, ps_qkv[:, 0:384],
                                            cos3_sb[:, mm, :], ALU.mult)
                    rh = s12.tile([P, 384], bf16, tag="rh")
                    rh_v = rh.rearrange("p (g x d) -> p g x d", g=3, x=2)
                    pv = ps_qkv[:, 0:384].rearrange("p (g x d) -> p g x d", g=3, x=2)
                    sin_v = sin3_sb[:, mm, :].rearrange("p (g x d) -> p g x d", g=3, x=2)
                    nc.vector.tensor_tensor(rh_v[:, :, 0, :], pv[:, :, 1, :],
                                            sin_v[:, :, 0, :], ALU.mult)
                    nc.vector.tensor_tensor(rh_v[:, :, 1, :], pv[:, :, 0, :],
                                            sin_v[:, :, 1, :], ALU.mult)
                    nc.gpsimd.tensor_tensor(ro, ro, rh, ALU.add)
                    # normalize all three heads in one broadcast multiply
                    nc.gpsimd.tensor_tensor(
                        qs_all[:, m, :].rearrange("p (g d) -> p g d", g=3),
                        ro.rearrange("p (g d) -> p g d", g=3),
                        rinv[:, :, None].to_broadcast((P, 3, D)), ALU.mult)
                    if m >= 4:
                        transpose_m(m - 4)
                for mt in range(M - 4, M):
                    transpose_m(mt)

                # prefetch the o-projection weights now that the high-priority
                # x / wqkv loads have been issued
                wo_src = woT.ap().rearrange("p (k f) -> p k f", k=KO)
                for k4 in range(0, KO, 4):
                    nc.sync.dma_start(wo_sb[:, k4:k4 + 4, :], wo_src[:, k4:k4 + 4, :])

            xin_cm.__exit__(None, None, None)
            wq_cm.__exit__(None, None, None)

            early = probe == "s12"
            if early:
                with tc.tile_pool(name="pr", bufs=2) as pr:
                    for (src, r0) in ((qT_sb[:, 0, 0:HS], 0), (kT_sb[:, 0:HS], P)):
                        ptile = pr.tile([P, HS], f32, tag="ptile")
                        nc.scalar.copy(ptile, src)
                        nc.sync.dma_start(out_d.ap()[r0:r0 + P, :], ptile)

            # ---------------- stage 3: causal attention, head-major so each
            # head's AllToAll overlaps the next head's compute
            with tc.tile_pool(name="s3", bufs=4) as s3, \
                 tc.tile_pool(name="s3b", bufs=2) as s3b, \
                 tc.tile_pool(name="ps3", bufs=2, space="PSUM") as ps3:
                # pre-zero the score PSUM buffers: diagonal chunks only write
                # the causally-live columns, the additive mask handles the rest
                sinit = [ps3.tile([P, 2, CW], f32, tag="s", bufs=3,
                                  name=f"sinit{i_}")
                         for i_ in range(3)]
                for t_ in sinit:
                    nc.vector.memset(t_[:], 0.0)
                for h in range(NQ if not early else 0):
                    for b in range(2):
                        for c in (3, 2, 1, 0):
                            qv = qT_sb[:, h, S * b + CW * c: S * b + CW * (c + 1)]
                            npairs = 2 * c + 2
                            o_ps = ps3.tile([P, CW], f32, tag="o", bufs=1)
                            sum_ps = ps3.tile([P, CW], f32, tag="sum", bufs=1)
                            for pp in range(npairs):
                                s_ps2 = ps3.tile([P, 2, CW], f32, tag="s", bufs=3)
                                pT2 = s3.tile([P, 2, CW], bf16, tag="pT")
                                diag = pp >= 2 * c
                                for i in range(2):
                                    kb = 2 * pp + i
                                    off = P * (kb - 4 * c) if diag else 0
                                    nc.tensor.matmul(
                                        s_ps2[:, i, off:CW],
                                        kT_sb[:, S * b + P * kb: S * b + P * (kb + 1)],
                                        qv[:, off:CW], start=True, stop=True)
                                if diag:
                                    jj = pp - 2 * c
                                    mk, ext = ((maskA_sb, 256) if jj == 0
                                               else (maskB_sb, 512))
                                    nc.vector.scalar_tensor_tensor(
                                        s_ps2[:, :, 0:ext], s_ps2[:, :, 0:ext],
                                        1.0, mk[:], ALU.mult, ALU.add)
                                nc.scalar.activation(pT2, s_ps2, AF.Exp)
                                for i in range(2):
                                    kb = 2 * pp + i
                                    off = P * (kb - 4 * c) if diag else 0
                                    nc.tensor.matmul(o_ps[:, off:CW],
                                                     v_sb[:, T * b + kb, :],
                                                     pT2[:, i, off:CW],
                                                     start=(pp == 0 and i == 0),
                                                     stop=(pp == npairs - 1 and i == 1))
                                    nc.tensor.matmul(sum_ps[:, off:CW], onesq_sb,
                                                     pT2[:, i, off:CW],
                                                     start=(pp == 0 and i == 0),
                                                     stop=(pp == npairs - 1 and i == 1))
                            rec = s3b.tile([P, CW], f32, tag="rec")
                            nc.vector.reciprocal_approx_fast(rec, sum_ps)
                            o_sb = s3b.tile([P, CW], bf16, tag="o_sb")
                            nc.vector.tensor_tensor(o_sb, o_ps, rec, ALU.mult)
                            r0 = P * (4 * b + c)
                            nc.sync.dma_start(a2a_in[h][r0:r0 + P, :], o_sb)
                    if probe == "full":
                        nc.gpsimd.collective_compute(
                            "AllToAll", mybir.AluOpType.bypass,
                            ins=[a2a_in[h][:].opt()], outs=[a2a_out[h][:].opt()],
                            replica_groups=[list(range(N_CORES))],
                        )
                        nc.sync.dma_start(
                            attn_sb[h][:],
                            a2a_out[h][:].rearrange("(k p) t -> p k t", p=P))
            if probe == "s3":
                a2a_out = a2a_in

            stageA.__exit__(None, None, None)

            # ---------------- stage 4: output projection for this core's strip
            with tc.tile_pool(name="s4", bufs=1) as s4, \
                 tc.tile_pool(name="s4o", bufs=2) as s4o, \
                 tc.tile_pool(name="ps4", bufs=2, space="PSUM") as ps4:
                if probe == "s3":
                    for h in range(NQ if not early else 0):
                        nc.sync.dma_start(
                            attn_sb[h][:],
                            a2a_out[h][:].rearrange("(k p) t -> p k t", p=P))
                accs = {}
                OCH = HS // 512
                for t in range(0 if early else CW // P):
                    for oc in range(OCH):
                        ps_o = ps4.tile([P, 512], f32, tag="oproj")
                        for k8 in range(KO // NQ):
                            nc.tensor.matmul(
                                ps_o, attn_sb[0][:, k8, P * t:P * (t + 1)],
                                wo_sb[:, NQ * k8, 512 * oc:512 * (oc + 1)],
                                start=(k8 == 0), stop=(k8 == KO // NQ - 1))
                        acc = s4.tile([P, 512], bf16, tag="acc", bufs=16)
                        nc.scalar.copy(acc, ps_o)
                        accs[(t, oc)] = acc
                for t in range(0 if early else CW // P):
                    for oc in range(OCH):
                        ps_o = ps4.tile([P, 512], f32, tag="oproj")
                        for k8 in range(KO // NQ):
                            nc.tensor.matmul(
                                ps_o, attn_sb[1][:, k8, P * t:P * (t + 1)],
                                wo_sb[:, NQ * k8 + 1, 512 * oc:512 * (oc + 1)],
                                start=(k8 == 0), stop=(k8 == KO // NQ - 1))
                        osb = s4o.tile([P, 512], f32, tag="osb")
                        nc.vector.scalar_tensor_tensor(
                            osb, ps_o, 1.0, accs[(t, oc)], ALU.mult, ALU.add)
                        nc.sync.dma_start(
                            out_d.ap()[P * t:P * (t + 1), 512 * oc:512 * (oc + 1)], osb)
            _wo_free()

    nc.compile()
    return nc


def shard_inputs(inputs, S=2048, HS=2048):
    """Full problem inputs -> list of 8 per-core in_maps (host-side prep)."""
    x = np.asarray(inputs["x"], np.float32)
    cos = np.asarray(inputs["cos"], np.float32)
    sin = np.asarray(inputs["sin"], np.float32)
    wq = np.asarray(inputs["wq"], np.float32)
    wk = np.asarray(inputs["wk"], np.float32)
    wv = np.asarray(inputs["wv"], np.float32)
    wo = np.asarray(inputs["wo"], np.float32)
    qw = np.asarray(inputs["q_norm_w"], np.float32)
    kw = np.asarray(inputs["k_norm_w"], np.float32)

    T = S // P
    M = 2 * T
    KT = HS // P

    xT_t = np.ascontiguousarray(
        x.reshape(M, P, KT, P).transpose(0, 3, 2, 1).reshape(M, P, HS)).astype(BF16)

    sgn = np.concatenate([-np.ones(64, np.float32), np.ones(64, np.float32)])
    kscale = 1.0 / np.sqrt(D)

    def tile_p(a):
        # [(n*P), inner] row-major -> [P, n*inner] partition-major
        n = a.shape[0] // P
        return np.ascontiguousarray(
            a.reshape(n, P, a.shape[1]).transpose(1, 0, 2).reshape(P, -1))

    def fold(w, s):
        w_rot = np.concatenate([w[64:], w[:64]])
        c = (cos * w[None, :] * s).astype(np.float32)           # [S, D]
        sn = (sin * (w_rot * sgn)[None, :] * s).astype(np.float32)
        return c, sn

    cq, sq_ = fold(qw, 1.0)
    ck, sk_ = fold(kw, kscale)
    cos3 = tile_p(np.concatenate([cq, cq, ck], axis=1)).astype(BF16)  # [P, T*384]
    sin3 = tile_p(np.concatenate([sq_, sq_, sk_], axis=1)).astype(BF16)

    # additive causal masks for the diagonal 4-block group of each chunk.
    # Block j of the group covers kv rows r at q columns t, masked iff
    # 128*j + r > t. Pair 0 = blocks (0,1) over cols [0,256); pair 1 =
    # blocks (2,3) over cols [0,512).
    r = np.arange(P)[:, None]

    def blockmask(j, width):
        t = np.arange(width)[None, :]
        return np.where(P * j + r > t, np.float32(NEGM), np.float32(0.0))

    maskA = np.stack([blockmask(0, 256), blockmask(1, 256)], axis=1)  # [P,2,256]
    maskB = np.stack([blockmask(2, 512), blockmask(3, 512)], axis=1)  # [P,2,512]
    maskA = np.ascontiguousarray(maskA.reshape(P, -1)).astype(np.float32)
    maskB = np.ascontiguousarray(maskB.reshape(P, -1)).astype(np.float32)

    onesq = np.ones((P, P), BF16)
    ident = np.eye(P, dtype=np.float32).astype(BF16)
    woT = tile_p(np.ascontiguousarray(wo.T)).astype(BF16)

    in_maps = []
    for c in range(N_CORES):
        kvh = c // 2
        wq_c = wq[2 * c * D:(2 * c + 2) * D]       # [256, HS]
        wk_c = wk[kvh * D:(kvh + 1) * D]           # [128, HS]
        wv_c = wv[kvh * D:(kvh + 1) * D]           # [128, HS]
        wqkv = np.concatenate([wq_c, wk_c, wv_c], axis=0)  # [512, HS]
        wqkvT = tile_p(np.ascontiguousarray(wqkv.T)).astype(BF16)
        in_maps.append({
            "xT": xT_t, "wqkvT": wqkvT, "woT": woT,
            "cos3": cos3, "sin3": sin3,
            "maskA": maskA, "maskB": maskB,
            "onesq": onesq, "ident": ident,
        })
    return in_maps


def assemble(outs, S=2048, HS=2048):
    """Per-core strip outputs -> full [B, S, HS] output."""
    CW = S // 4
    full = np.empty((B, S, HS), np.float32)
    for c in range(N_CORES):
        full[c // 4, (c % 4) * CW:(c % 4 + 1) * CW, :] = outs[c]
    return full


_CACHE = {}


def _get_compiled(S=2048, HS=2048, probe="full"):
    key = (S, HS, probe)
    if key not in _CACHE:
        _CACHE[key] = build(S, HS, probe)
    return _CACHE[key]


def _ensure_ntff_hook():
    """The image's antenv lacks axon_hooks; synthesize it so trace=True works."""
    import types
    try:
        from antenv.axon_hooks import get_axon_ntff_profile_hook  # noqa: F401
        return
    except ImportError:
        pass
    import antenv
    from trn_agent_boot.trn_boot import _ntff_profile_via_ctypes
    mod = types.ModuleType("antenv.axon_hooks")
    mod._hook = _ntff_profile_via_ctypes("/opt/axon/libaxon_pjrt.so")
    mod.set_axon_ntff_profile_hook = lambda h: setattr(mod, "_hook", h)
    mod.get_axon_ntff_profile_hook = lambda: mod._hook
    sys.modules["antenv.axon_hooks"] = mod
    antenv.axon_hooks = mod


def run(inputs, S=2048, HS=2048, trace=False, tmpdir=None, probe="full"):
    import concourse.bass_utils as bu
    if trace:
        _ensure_ntff_hook()
        bu.upload_artifacts = lambda d: ""  # no artifact bucket in this container
    nc = _get_compiled(S, HS, probe)
    in_maps = shard_inputs(inputs, S, HS)
    res = bu.run_bass_kernel_spmd(nc, in_maps, core_ids=list(range(N_CORES)),
                                  trace=trace, tmpdir=tmpdir)
    out = assemble([r["out"] for r in res.results], S, HS)
    return out, res.exec_time_ns


def kernel(**inputs):
    out, _ = run(inputs)
    return out


# revision 32
# speedup vs baseline: 1.0272x; 1.0272x over previous
"""Trainium2 Bass kernel for GQA attention block (B=2, S=2048, HS=2048, H=16, HKV=4, D=128).

Strategy (8 NeuronCores, SPMD):
  - Head-parallel: core c computes q-heads {2c, 2c+1} and kv-head c//2 for BOTH batches.
  - bf16 matmuls throughout (fp8 fails the 2e-2 accuracy gate: random-sign dot
    products keep the ~3% element quantization error instead of averaging it).
  - RMS-norm + RoPE computed straight from PSUM: RoPE first (rotation preserves
    norms), then a single broadcast multiply by rsqrt(mean-square). Work spread
    across Vector/Scalar/GpSimd so no single engine saturates.
  - Causal flash attention in transposed layout: S^T = K_rope @ Q_rope^T ([kv, q]),
    additive causal masks applied in PSUM before exp, exp on ScalarE over two
    kv-blocks at once, diagonal-group QK matmuls skip fully-masked columns,
    O^T = V^T @ P^T and denominators via ones-matmul accumulated in PSUM.
  - One 8-rank AllToAll per q-head redistributes head-shards -> (batch, seq-strip)
    shards; head 0's collective overlaps head 1's attention.
  - Output projection per strip with a fused add epilogue; host concatenates strips.
"""

import sys

sys.path.insert(0, "/opt/trn_rl_repo")

import numpy as np
import ml_dtypes

BF16 = ml_dtypes.bfloat16

B, H, HKV, D = 2, 16, 4, 128
EPS = 1e-6
P = 128
N_CORES = 8
NEGM = -60.0


def build(S=2048, HS=2048, probe="full"):
    """Build + compile the SPMD graph. Returns the Bacc module."""
    import concourse.bacc as bacc
    import concourse.tile as tile
    import concourse.mybir as mybir

    dt = mybir.dt
    f32 = dt.float32
    bf16 = dt.bfloat16
    AF = mybir.ActivationFunctionType
    ALU = mybir.AluOpType

    T = S // P          # tok tiles per batch (16)
    M = 2 * T           # tok tiles total (2 batches)
    KT = HS // P        # contraction tiles for qkv projection (16)
    KO = (H * D) // P   # contraction tiles for o projection (16)
    CW = S // 4         # q-chunk width == strip width (512)
    CB = CW // P        # kv blocks per chunk step (4)
    NQ = 2              # q heads per core

    nc = bacc.Bacc("TRN2", target_bir_lowering=False, debug=False,
                   enable_asserts=True, num_devices=N_CORES)

    xT = nc.dram_tensor("xT", [M, P, KT * P], bf16, kind="ExternalInput")
    wqkvT = nc.dram_tensor("wqkvT", [P, KT * 512], bf16, kind="ExternalInput")
    woT = nc.dram_tensor("woT", [P, KO * HS], bf16, kind="ExternalInput")
    cos3_d = nc.dram_tensor("cos3", [P, T * 384], bf16, kind="ExternalInput")
    sin3_d = nc.dram_tensor("sin3", [P, T * 384], bf16, kind="ExternalInput")
    maskA_d = nc.dram_tensor("maskA", [P, 2 * 256], f32, kind="ExternalInput")
    maskB_d = nc.dram_tensor("maskB", [P, 2 * 512], f32, kind="ExternalInput")
    onesq_d = nc.dram_tensor("onesq", [P, P], bf16, kind="ExternalInput")
    ident_d = nc.dram_tensor("ident", [P, P], bf16, kind="ExternalInput")
    out_d = nc.dram_tensor("out", [CW, HS], f32, kind="ExternalOutput")

    with tile.TileContext(nc) as tc:
        with tc.tile_pool(name="const", bufs=1) as cpool, \
             tc.tile_pool(name="dram", bufs=1, space="DRAM") as dpool:

            # o-projection weights: allocated up front (outlives stageA pools);
            # the DMAs are issued after stage 1+2 so they don't delay the
            # x / wqkv loads feeding the first matmuls
            wo_sb, _wo_free = tc.tile([P, KO, HS], bf16, name="wo_sb")

            # pools that live only through stages 1-3 (freed before o-proj)
            stageA = tc.tile_pool(name="stageA", bufs=1)
            qkvpool = stageA.__enter__()
            wq_cm = tc.tile_pool(name="wqp", bufs=1)
            wqpool = wq_cm.__enter__()
            xin_cm = tc.tile_pool(name="xin", bufs=4)
            xin = xin_cm.__enter__()

            xms = {}

            def load_xm(m):
                t_ = xin.tile([P, KT, P], bf16, tag="xm", name=f"xm{m}")
                src = xT.ap()[m].rearrange("p (k t) -> p k t", k=KT)
                if m == 0:
                    # finer pieces so the first matmuls can start sooner
                    for k4 in range(0, KT, 4):
                        nc.sync.dma_start(t_[:, k4:k4 + 4, :], src[:, k4:k4 + 4, :])
                else:
                    nc.sync.dma_start(t_[:], src)
                xms[m] = t_

            wqkv_sb = wqpool.tile([P, KT, 512], bf16, name="wqkv_sb")
            wq_src = wqkvT.ap().rearrange("p (k f) -> p k f", k=KT)
            load_xm(0)
            nc.sync.dma_start(wqkv_sb[:, 0:4, :], wq_src[:, 0:4, :])
            load_xm(1)
            for k4 in range(4, KT, 4):
                nc.sync.dma_start(wqkv_sb[:, k4:k4 + 4, :], wq_src[:, k4:k4 + 4, :])
            load_xm(2)

            cos3_sb = cpool.tile([P, T, 384], bf16, name="cos3_sb")
            sin3_sb = cpool.tile([P, T, 384], bf16, name="sin3_sb")
            nc.sync.dma_start(cos3_sb[:], cos3_d.ap().rearrange("p (t d) -> p t d", t=T))
            nc.sync.dma_start(sin3_sb[:], sin3_d.ap().rearrange("p (t d) -> p t d", t=T))
            maskA_sb = cpool.tile([P, 2, 256], f32, name="maskA_sb")
            nc.sync.dma_start(maskA_sb[:], maskA_d.ap().rearrange("p (i t) -> p i t", i=2))
            maskB_sb = cpool.tile([P, 2, 512], f32, name="maskB_sb")
            nc.sync.dma_start(maskB_sb[:], maskB_d.ap().rearrange("p (i t) -> p i t", i=2))
            onesq_sb = cpool.tile([P, P], bf16, name="onesq_sb")
            nc.sync.dma_start(onesq_sb[:], onesq_d.ap())
            ident_sb = cpool.tile([P, P], bf16, name="ident_sb")
            nc.sync.dma_start(ident_sb[:], ident_d.ap())
            eps_sb = cpool.tile([P, 1], f32, name="eps_sb")
            nc.gpsimd.memset(eps_sb[:], EPS)

            a2a_in = [dpool.tile([1024, CW], bf16, name=f"a2a_in{h}")
                      for h in range(NQ)]
            a2a_out = [dpool.tile([1024, CW], bf16, name=f"a2a_out{h}")
                       for h in range(NQ)]

            qT_sb = qkvpool.tile([P, NQ, 2 * S], bf16, name="qT_sb")
            kT_sb = qkvpool.tile([P, 2 * S], bf16, name="kT_sb")
            v_sb = qkvpool.tile([P, M, D], bf16, name="v_sb")
            qs_all = qkvpool.tile([P, M, 384], bf16, name="qs_all")

            attn_sb = [cpool.tile([P, KO // NQ, CW], bf16, name=f"attn_sb{h}")
                       for h in range(NQ)]

            # ---------------- stage 1+2: QKV projection, RoPE, RMS norm, transpose
            with tc.tile_pool(name="s12", bufs=3) as s12, \
                 tc.tile_pool(name="ps12", bufs=3, space="PSUM") as ps12:

                def transpose_m(mt):
                    # transposes of qs_all[mt] interleave with later m's QKV
                    # matmuls; their input has been ready for several
                    # iterations so the PE never stalls on them
                    col = P * mt if mt < T else S + P * (mt % T)
                    tp3 = ps12.tile([P, 3, P], bf16, tag="tp", name=f"tp{mt}")
                    for idx in range(3):
                        nc.tensor.transpose(tp3[:, idx, :],
                                            qs_all[:, mt, idx * P:(idx + 1) * P],
                                            ident_sb)
                    nc.vector.tensor_scalar_add(
                        qT_sb[:, 0:2, col:col + P], tp3[:, 0:2, :], 0.0)
                    nc.scalar.copy(kT_sb[:, col:col + P], tp3[:, 2, :])

                for m in range(M):
                    mm = m % T
                    if m + 3 < M:
                        load_xm(m + 3)
                    xm = xms.pop(m)
                    ps_qkv = ps12.tile([P, 512], f32, tag="qkv")
                    for k in range(KT):
                        nc.tensor.matmul(ps_qkv, xm[:, k, :], wqkv_sb[:, k, :],
                                         start=(k == 0), stop=(k == KT - 1))
                    # V: copy to [tok, d] layout
                    nc.scalar.copy(v_sb[:, m, :], ps_qkv[:, 384:512])
                    # sum of squares per head (q0|q1|k) for RMS norm
                    sq = s12.tile([P, 384], bf16, tag="sq")
                    nc.scalar.square(sq, ps_qkv[:, 0:384])
                    ssum = s12.tile([P, 3], f32, tag="ssum")
                    nc.vector.tensor_reduce(
                        ssum, sq.rearrange("p (g d) -> p g d", g=3),
                        mybir.AxisListType.X, ALU.add)
                    rms = s12.tile([P, 3], f32, tag="rms")
                    nc.scalar.activation(rms, ssum, AF.Sqrt,
                                         bias=eps_sb[:], scale=1.0 / D)
                    rinv = s12.tile([P, 3], f32, tag="rinv")
                    nc.vector.reciprocal_approx_fast(rinv, rms)
                    # RoPE on raw PSUM values (rotation commutes with the norm)
                    ro = s12.tile([P, 384], bf16, tag="ro")
                    nc.vector.tensor_tensor(ro, ps_qkv[:, 0:384],
                                            cos3_sb[:, mm, :], ALU.mult)
                    rh = s12.tile([P, 384], bf16, tag="rh")
                    rh_v = rh.rearrange("p (g x d) -> p g x d", g=3, x=2)
                    pv = ps_qkv[:, 0:384].rearrange("p (g x d) -> p g x d", g=3, x=2)
                    sin_v = sin3_sb[:, mm, :].rearrange("p (g x d) -> p g x d", g=3, x=2)
                    nc.vector.tensor_tensor(rh_v[:, :, 0, :], pv[:, :, 1, :],
                                            sin_v[:, :, 0, :], ALU.mult)
                    nc.vector.tensor_tensor(rh_v[:, :, 1, :], pv[:, :, 0, :],
                                            sin_v[:, :, 1, :], ALU.mult)
                    nc.gpsimd.tensor_tensor(ro, ro, rh, ALU.add)
                    # normalize all three heads in one broadcast multiply
                    nc.gpsimd.tensor_tensor(
                        qs_all[:, m, :].rearrange("p (g d) -> p g d", g=3),
                        ro.rearrange("p (g d) -> p g d", g=3),
                        rinv[:, :, None].to_broadcast((P, 3, D)), ALU.mult)
                    if m >= 4:
                        transpose_m(m - 4)
                for mt in range(M - 4, M):
                    transpose_m(mt)

                # prefetch the o-projection weights now that the high-priority
                # x / wqkv loads have been issued
                wo_src = woT.ap().rearrange("p (k f) -> p k f", k=KO)
                for k4 in range(0, KO, 4):
                    nc.sync.dma_start(wo_sb[:, k4:k4 + 4, :], wo_src[:, k4:k4 + 4, :])

            xin_cm.__exit__(None, None, None)
            wq_cm.__exit__(None, None, None)

            early = probe == "s12"
            if early:
                with tc.tile_pool(name="pr", bufs=2) as pr:
                    for (src, r0) in ((qT_sb[:, 0, 0:HS], 0), (kT_sb[:, 0:HS], P)):
                        ptile = pr.tile([P, HS], f32, tag="ptile")
                        nc.scalar.copy(ptile, src)
                        nc.sync.dma_start(out_d.ap()[r0:r0 + P, :], ptile)

            # ---------------- stage 3: causal attention, head-major so each
            # head's AllToAll overlaps the next head's compute
            with tc.tile_pool(name="s3", bufs=4) as s3, \
                 tc.tile_pool(name="s3b", bufs=2) as s3b, \
                 tc.tile_pool(name="ps3", bufs=2, space="PSUM") as ps3:
                # pre-zero the score PSUM buffers: diagonal chunks only write
                # the causally-live columns, the additive mask handles the rest
                sinit = [ps3.tile([P, 2, CW], f32, tag="s", bufs=3,
                                  name=f"sinit{i_}")
                         for i_ in range(3)]
                for t_ in sinit:
                    nc.vector.memset(t_[:], 0.0)
                for h in range(NQ if not early else 0):
                    for b in range(2):
                        for c in (3, 2, 1, 0):
                            qv = qT_sb[:, h, S * b + CW * c: S * b + CW * (c + 1)]
                            npairs = 2 * c + 2
                            o_ps = ps3.tile([P, CW], f32, tag="o", bufs=1)
                            sum_ps = ps3.tile([P, CW], f32, tag="sum", bufs=1)
                            for pp in range(npairs):
                                s_ps2 = ps3.tile([P, 2, CW], f32, tag="s", bufs=3)
                                pT2 = s3.tile([P, 2, CW], bf16, tag="pT")
                                diag = pp >= 2 * c
                                for i in range(2):
                                    kb = 2 * pp + i
                                    off = P * (kb - 4 * c) if diag else 0
                                    nc.tensor.matmul(
                                        s_ps2[:, i, off:CW],
                                        kT_sb[:, S * b + P * kb: S * b + P * (kb + 1)],
                                        qv[:, off:CW], start=True, stop=True)
                                if diag:
                                    jj = pp - 2 * c
                                    mk, ext = ((maskA_sb, 256) if jj == 0
                                               else (maskB_sb, 512))
                                    nc.vector.scalar_tensor_tensor(
                                        s_ps2[:, :, 0:ext], s_ps2[:, :, 0:ext],
                                        1.0, mk[:], ALU.mult, ALU.add)
                                nc.scalar.activation(pT2, s_ps2, AF.Exp)
                                for i in range(2):
                                    kb = 2 * pp + i
                                    off = P * (kb - 4 * c) if diag else 0
                                    nc.tensor.matmul(o_ps[:, off:CW],
                                                     v_sb[:, T * b + kb, :],
                                                     pT2[:, i, off:CW],
                                                     start=(pp == 0 and i == 0),
                                                     stop=(pp == npairs - 1 and i == 1))
                                    nc.tensor.matmul(sum_ps[:, off:CW], onesq_sb,
                                                     pT2[:, i, off:CW],
                                                     start=(pp == 0 and i == 0),
                                                     stop=(pp == npairs - 1 and i == 1))
                            rec = s3b.tile([P, CW], f32, tag="rec")
                            nc.vector.reciprocal_approx_fast(rec, sum_ps)
                            o_sb = s3b.tile([P, CW], bf16, tag="o_sb", bufs=4)
                            nc.vector.tensor_tensor(o_sb, o_ps, rec, ALU.mult)
                            r0 = P * (4 * b + c)
                            nc.sync.dma_start(a2a_in[h][r0:r0 + P, :], o_sb)
                    if probe == "full":
                        nc.gpsimd.collective_compute(
                            "AllToAll", mybir.AluOpType.bypass,
                            ins=[a2a_in[h][:].opt()], outs=[a2a_out[h][:].opt()],
                            replica_groups=[list(range(N_CORES))],
                        )
                        a2a_v = a2a_out[h][:].rearrange("(k p) t -> p k t", p=P)
                        for k2 in range(0, KO // NQ, 2):
                            nc.sync.dma_start(attn_sb[h][:, k2:k2 + 2, :],
                                              a2a_v[:, k2:k2 + 2, :])
            if probe == "s3":
                a2a_out = a2a_in

            stageA.__exit__(None, None, None)

            # ---------------- stage 4: output projection for this core's strip
            with tc.tile_pool(name="s4", bufs=1) as s4, \
                 tc.tile_pool(name="s4o", bufs=2) as s4o, \
                 tc.tile_pool(name="ps4", bufs=2, space="PSUM") as ps4:
                if probe == "s3":
                    for h in range(NQ if not early else 0):
                        nc.sync.dma_start(
                            attn_sb[h][:],
                            a2a_out[h][:].rearrange("(k p) t -> p k t", p=P))
                accs = {}
                OCH = HS // 512
                for t in range(0 if early else CW // P):
                    for oc in range(OCH):
                        ps_o = ps4.tile([P, 512], f32, tag="oproj")
                        for k8 in range(KO // NQ):
                            nc.tensor.matmul(
                                ps_o, attn_sb[0][:, k8, P * t:P * (t + 1)],
                                wo_sb[:, NQ * k8, 512 * oc:512 * (oc + 1)],
                                start=(k8 == 0), stop=(k8 == KO // NQ - 1))
                        acc = s4.tile([P, 512], bf16, tag="acc", bufs=16)
                        nc.scalar.copy(acc, ps_o)
                        accs[(t, oc)] = acc
                for t in range(0 if early else CW // P):
                    for oc in range(OCH):
                        ps_o = ps4.tile([P, 512], f32, tag="oproj")
                        for k8 in range(KO // NQ):
                            nc.tensor.matmul(
                                ps_o, attn_sb[1][:, k8, P * t:P * (t + 1)],
                                wo_sb[:, NQ * k8 + 1, 512 * oc:512 * (oc + 1)],
                                start=(k8 == 0), stop=(k8 == KO // NQ - 1))
                        osb = s4o.tile([P, 512], f32, tag="osb")
                        nc.vector.scalar_tensor_tensor(
                            osb, ps_o, 1.0, accs[(t, oc)], ALU.mult, ALU.add)
                        nc.sync.dma_start(
                            out_d.ap()[P * t:P * (t + 1), 512 * oc:512 * (oc + 1)], osb)
            _wo_free()

    nc.compile()
    return nc


def shard_inputs(inputs, S=2048, HS=2048):
    """Full problem inputs -> list of 8 per-core in_maps (host-side prep)."""
    x = np.asarray(inputs["x"], np.float32)
    cos = np.asarray(inputs["cos"], np.float32)
    sin = np.asarray(inputs["sin"], np.float32)
    wq = np.asarray(inputs["wq"], np.float32)
    wk = np.asarray(inputs["wk"], np.float32)
    wv = np.asarray(inputs["wv"], np.float32)
    wo = np.asarray(inputs["wo"], np.float32)
    qw = np.asarray(inputs["q_norm_w"], np.float32)
    kw = np.asarray(inputs["k_norm_w"], np.float32)

    T = S // P
    M = 2 * T
    KT = HS // P

    xT_t = np.ascontiguousarray(
        x.reshape(M, P, KT, P).transpose(0, 3, 2, 1).reshape(M, P, HS)).astype(BF16)

    sgn = np.concatenate([-np.ones(64, np.float32), np.ones(64, np.float32)])
    kscale = 1.0 / np.sqrt(D)

    def tile_p(a):
        # [(n*P), inner] row-major -> [P, n*inner] partition-major
        n = a.shape[0] // P
        return np.ascontiguousarray(
            a.reshape(n, P, a.shape[1]).transpose(1, 0, 2).reshape(P, -1))

    def fold(w, s):
        w_rot = np.concatenate([w[64:], w[:64]])
        c = (cos * w[None, :] * s).astype(np.float32)           # [S, D]
        sn = (sin * (w_rot * sgn)[None, :] * s).astype(np.float32)
        return c, sn

    cq, sq_ = fold(qw, 1.0)
    ck, sk_ = fold(kw, kscale)
    cos3 = tile_p(np.concatenate([cq, cq, ck], axis=1)).astype(BF16)  # [P, T*384]
    sin3 = tile_p(np.concatenate([sq_, sq_, sk_], axis=1)).astype(BF16)

    # additive causal masks for the diagonal 4-block group of each chunk.
    # Block j of the group covers kv rows r at q columns t, masked iff
    # 128*j + r > t. Pair 0 = blocks (0,1) over cols [0,256); pair 1 =
    # blocks (2,3) over cols [0,512).
    r = np.arange(P)[:, None]

    def blockmask(j, width):
        t = np.arange(width)[None, :]
        return np.where(P * j + r > t, np.float32(NEGM), np.float32(0.0))

    maskA = np.stack([blockmask(0, 256), blockmask(1, 256)], axis=1)  # [P,2,256]
    maskB = np.stack([blockmask(2, 512), blockmask(3, 512)], axis=1)  # [P,2,512]
    maskA = np.ascontiguousarray(maskA.reshape(P, -1)).astype(np.float32)
    maskB = np.ascontiguousarray(maskB.reshape(P, -1)).astype(np.float32)

    onesq = np.ones((P, P), BF16)
    ident = np.eye(P, dtype=np.float32).astype(BF16)
    woT = tile_p(np.ascontiguousarray(wo.T)).astype(BF16)

    in_maps = []
    for c in range(N_CORES):
        kvh = c // 2
        wq_c = wq[2 * c * D:(2 * c + 2) * D]       # [256, HS]
        wk_c = wk[kvh * D:(kvh + 1) * D]           # [128, HS]
        wv_c = wv[kvh * D:(kvh + 1) * D]           # [128, HS]
        wqkv = np.concatenate([wq_c, wk_c, wv_c], axis=0)  # [512, HS]
        wqkvT = tile_p(np.ascontiguousarray(wqkv.T)).astype(BF16)
        in_maps.append({
            "xT": xT_t, "wqkvT": wqkvT, "woT": woT,
            "cos3": cos3, "sin3": sin3,
            "maskA": maskA, "maskB": maskB,
            "onesq": onesq, "ident": ident,
        })
    return in_maps


def assemble(outs, S=2048, HS=2048):
    """Per-core strip outputs -> full [B, S, HS] output."""
    CW = S // 4
    full = np.empty((B, S, HS), np.float32)
    for c in range(N_CORES):
        full[c // 4, (c % 4) * CW:(c % 4 + 1) * CW, :] = outs[c]
    return full


_CACHE = {}


def _get_compiled(S=2048, HS=2048, probe="full"):
    key = (S, HS, probe)
    if key not in _CACHE:
        _CACHE[key] = build(S, HS, probe)
    return _CACHE[key]


def _ensure_ntff_hook():
    """The image's antenv lacks axon_hooks; synthesize it so trace=True works."""
    import types
    try:
        from antenv.axon_hooks import get_axon_ntff_profile_hook  # noqa: F401
        return
    except ImportError:
        pass
    import antenv
    from trn_agent_boot.trn_boot import _ntff_profile_via_ctypes
    mod = types.ModuleType("antenv.axon_hooks")
    mod._hook = _ntff_profile_via_ctypes("/opt/axon/libaxon_pjrt.so")
    mod.set_axon_ntff_profile_hook = lambda h: setattr(mod, "_hook", h)
    mod.get_axon_ntff_profile_hook = lambda: mod._hook
    sys.modules["antenv.axon_hooks"] = mod
    antenv.axon_hooks = mod


def run(inputs, S=2048, HS=2048, trace=False, tmpdir=None, probe="full"):
    import concourse.bass_utils as bu
    if trace:
        _ensure_ntff_hook()
        bu.upload_artifacts = lambda d: ""  # no artifact bucket in this container
    nc = _get_compiled(S, HS, probe)
    in_maps = shard_inputs(inputs, S, HS)
    res = bu.run_bass_kernel_spmd(nc, in_maps, core_ids=list(range(N_CORES)),
                                  trace=trace, tmpdir=tmpdir)
    out = assemble([r["out"] for r in res.results], S, HS)
    return out, res.exec_time_ns


def kernel(**inputs):
    out, _ = run(inputs)
    return out
